# revision 19
# baseline (speedup 1.0000x reference)
"""Multi-head causal attention (B=2, S=2048, D=1024, H=16) on 8 TRN2 cores.

Sharding: core = (batch, group-of-4-heads). Each core computes attention for
its 4 heads of its batch and a rank-256 partial of the output projection;
the host sums the 4 partials per batch. The interleaved head split of the
reference (head h = columns h::16) is undone on the host by permuting the
weight matrices, so on-chip everything is head-contiguous.
"""
import sys
sys.path.insert(0, '/opt/trn_rl_repo')

import numpy as np

DIM = 1024
HEADS = 16
S = 2048
B = 2
HD = 64
N_CORES = 8
HPC = 4          # heads per core
PAIRS = 2        # processed as 2 pairs of heads (pair packs the 128-wide PE)
QCH = 512        # q chunk
NKT = S // 128   # k tiles per sequence

_nc_cache = None


def _build(debug=False):
    import concourse.tile as tile
    import concourse.mybir as mybir
    from concourse import bacc
    from concourse.masks import make_identity
    from contextlib import ExitStack

    f32 = mybir.dt.float32
    f32r = mybir.dt.float32r
    Exp = mybir.ActivationFunctionType.Exp

    nc = bacc.Bacc("TRN2", target_bir_lowering=False, debug=False,
                   enable_asserts=False, num_devices=N_CORES)

    xT = nc.dram_tensor("xT", [DIM, S], f32r, kind="ExternalInput").ap()
    qw = nc.dram_tensor("qw", [DIM, 256], f32r, kind="ExternalInput").ap()
    kw = nc.dram_tensor("kw", [DIM, 256], f32r, kind="ExternalInput").ap()
    vw = nc.dram_tensor("vw", [DIM, 256], f32r, kind="ExternalInput").ap()
    ow = nc.dram_tensor("ow", [256, DIM], f32r, kind="ExternalInput").ap()
    masks = nc.dram_tensor("masks", [4, 128, QCH], f32, kind="ExternalInput").ap()
    out = nc.dram_tensor("out", [S, DIM], f32, kind="ExternalOutput").ap()
    if debug:
        dbg = {}
        for name, shape, dt_ in (
                ("dQT0", [128, S], f32r), ("dKT0", [128, S], f32r),
                ("dVT0", [128, S], f32), ("dVaug0", [128, NKT, HD + 1], f32r),
                ("dAT0", [128, S], f32r), ("dP0", [128, QCH], f32r),
                ("dS0", [128, QCH], f32), ("dO0", [HD + 1, QCH], f32),
                ("dR0", [128, QCH], f32)):
            dbg[name] = nc.dram_tensor(name, shape, dt_, kind="ExternalOutput").ap()

    with tile.TileContext(nc) as tc, ExitStack() as ctx:
        const_pool = ctx.enter_context(tc.tile_pool(name="const", bufs=1))
        xin_pool = ctx.enter_context(tc.tile_pool(name="xin", bufs=2))
        big_pool = ctx.enter_context(tc.tile_pool(name="big", bufs=1))
        pt_pool = ctx.enter_context(tc.tile_pool(name="pt", bufs=4))
        small_pool = ctx.enter_context(tc.tile_pool(name="small", bufs=4))
        outst_pool = ctx.enter_context(tc.tile_pool(name="outst", bufs=4))
        psum_mm = ctx.enter_context(tc.tile_pool(name="psum_mm", bufs=4, space="PSUM"))
        psum_o = ctx.enter_context(tc.tile_pool(name="psum_o", bufs=3, space="PSUM"))

        # constants
        qw_sb = const_pool.tile([128, 8, 256], f32r, tag="qw")
        kw_sb = const_pool.tile([128, 8, 256], f32r, tag="kw")
        vw_sb = const_pool.tile([128, 8, 256], f32r, tag="vw")
        nc.sync.dma_start(out=qw_sb, in_=qw.rearrange("(kt p) m -> p kt m", p=128))
        nc.sync.dma_start(out=kw_sb, in_=kw.rearrange("(kt p) m -> p kt m", p=128))
        nc.sync.dma_start(out=vw_sb, in_=vw.rearrange("(kt p) m -> p kt m", p=128))
        ow_sb = const_pool.tile([128, 2, DIM], f32r, tag="ow")
        nc.sync.dma_start(out=ow_sb, in_=ow.rearrange("(t p) n -> p t n", p=128))
        masks_sb = const_pool.tile([128, 4, QCH], f32, tag="masks")
        nc.sync.dma_start(out=masks_sb, in_=masks.rearrange("i p q -> p i q"))
        ident = const_pool.tile([128, 128], f32, tag="ident")
        make_identity(nc, ident)

        QT = [big_pool.tile([128, S], f32r, tag=f"QT{p}", name=f"QT{p}") for p in range(PAIRS)]
        KT = [big_pool.tile([128, S], f32r, tag=f"KT{p}", name=f"KT{p}") for p in range(PAIRS)]
        VT = [big_pool.tile([128, S], f32, tag=f"VT{p}", name=f"VT{p}") for p in range(PAIRS)]
        AT = [big_pool.tile([128, S], f32r, tag=f"AT{p}", name=f"AT{p}") for p in range(PAIRS)]
        Vaug = [big_pool.tile([128, NKT, HD + 1], f32r, tag=f"Vaug{h}", name=f"Vaug{h}")
                for h in range(HPC)]
        ones_c = const_pool.tile([128, NKT, 1], f32, tag="ones")
        nc.vector.memset(ones_c, 1.0)
        for h in range(HPC):
            nc.vector.tensor_copy(out=Vaug[h][:, :, HD:HD + 1], in_=ones_c)
        Ef = const_pool.tile([33, 128], f32, tag="Ef")
        nc.vector.memset(Ef, 0.0)
        nc.vector.memset(Ef[0:1, 0:64], 1.0)
        nc.vector.memset(Ef[32:33, 64:128], 1.0)


        # ---- Phase 1: QKV projections (QT/KT/VT = W.T @ x.T, head-major) ----
        for j in range(S // QCH):
            qsl = slice(j * QCH, (j + 1) * QCH)
            xt = xin_pool.tile([128, 8, QCH], f32r, tag="xt")
            nc.sync.dma_start(
                out=xt,
                in_=xT.rearrange("(kt p) n -> p kt n", p=128)[:, :, qsl])
            for wsb, dstT in ((qw_sb, QT), (kw_sb, KT), (vw_sb, VT)):
                for ct in range(PAIRS):
                    ps = psum_mm.tile([128, QCH], f32, tag="mm")
                    for kt in range(8):
                        nc.tensor.matmul(
                            ps,
                            wsb[:, kt, ct * 128:(ct + 1) * 128],
                            xt[:, kt, :],
                            start=(kt == 0), stop=(kt == 7))
                    nc.vector.tensor_copy(out=dstT[ct][:, qsl], in_=ps)

        if debug:
            nc.sync.dma_start(out=dbg["dQT0"], in_=QT[0])
            nc.sync.dma_start(out=dbg["dKT0"], in_=KT[0])
            nc.sync.dma_start(out=dbg["dVT0"], in_=VT[0])

        # ---- Phase 1b: V -> Vaug (transpose to [k, hd] + ones column) ----
        for p in range(PAIRS):
            for kt in range(NKT):
                pst = psum_mm.tile([128, 128], f32, tag="mm")
                nc.tensor.transpose(pst, VT[p][:, kt * 128:(kt + 1) * 128], ident)
                for hh in range(2):
                    nc.vector.tensor_copy(
                        out=Vaug[2 * p + hh][:, kt, 0:HD],
                        in_=pst[:, hh * 64:(hh + 1) * 64])

        if debug:
            nc.sync.dma_start(out=dbg["dVaug0"], in_=Vaug[0])

        # ---- Phase 2: causal attention, pair-packed; then output proj ----
        for p in range(PAIRS):
            hA, hB = 2 * p, 2 * p + 1
            for j in range(S // QCH):
                nkt = 4 * (j + 1)
                qsl = slice(j * QCH, (j + 1) * QCH)
                oA = psum_o.tile([HD + 1, QCH], f32, tag="o")
                oB = psum_o.tile([HD + 1, QCH], f32, tag="o")
                for kt in range(nkt):
                    ksl = slice(kt * 128, (kt + 1) * 128)
                    sA = psum_mm.tile([128, QCH], f32, tag="mm")
                    sB = psum_mm.tile([128, QCH], f32, tag="mm")
                    nc.tensor.matmul(sA, KT[p][0:64, ksl],
                                     QT[p][0:64, qsl],
                                     start=True, stop=True)
                    nc.tensor.matmul(sB, KT[p][64:128, ksl],
                                     QT[p][64:128, qsl],
                                     start=True, stop=True)
                    pA = pt_pool.tile([128, QCH], f32r, tag="pt")
                    pB = pt_pool.tile([128, QCH], f32r, tag="pt")
                    nc.scalar.activation(out=pA, in_=sA, func=Exp)
                    nc.scalar.activation(out=pB, in_=sB, func=Exp)
                    if debug and p == 0 and j == 0 and kt == 0:
                        sstage = small_pool.tile([128, QCH], f32, tag="dbgs")
                        nc.vector.tensor_copy(out=sstage, in_=sA)
                        nc.sync.dma_start(out=dbg["dS0"], in_=sstage)
                    di = kt - 4 * j
                    if di >= 0:  # diagonal tile: apply causal mask
                        nc.vector.tensor_mul(pA, pA, masks_sb[:, di, :])
                        nc.vector.tensor_mul(pB, pB, masks_sb[:, di, :])
                    if debug and p == 0 and j == 0 and kt == 0:
                        nc.sync.dma_start(out=dbg["dP0"], in_=pA)
                    nc.tensor.matmul(oA, Vaug[hA][:, kt, :],
                                     pA,
                                     start=(kt == 0), stop=(kt == nkt - 1))
                    nc.tensor.matmul(oB, Vaug[hB][:, kt, :],
                                     pB,
                                     start=(kt == 0), stop=(kt == nkt - 1))
                # normalize: A = O / l  (l = ones-row of oX), store transposed
                rAB = small_pool.tile([33, QCH], f32, tag="r")
                nc.vector.memset(rAB, 0.0)
                with nc.allow_low_precision(reason="recip rows"):
                    nc.vector.reciprocal(rAB[0:1, :], oA[HD:HD + 1, :])
                    nc.vector.reciprocal(rAB[32:33, :], oB[HD:HD + 1, :])
                R = psum_mm.tile([128, QCH], f32, tag="mm")
                nc.tensor.matmul(R, Ef, rAB, start=True, stop=True)
                if debug and p == 0 and j == 0:
                    ostage = small_pool.tile([HD + 1, QCH], f32, tag="dbgo")
                    nc.vector.tensor_copy(out=ostage, in_=oA)
                    nc.sync.dma_start(out=dbg["dO0"], in_=ostage)
                    rstage = small_pool.tile([128, QCH], f32, tag="dbgR")
                    nc.vector.tensor_copy(out=rstage, in_=R)
                    nc.sync.dma_start(out=dbg["dR0"], in_=rstage)
                nc.vector.tensor_copy(out=AT[p][0:64, qsl], in_=oA[0:HD, :])
                nc.vector.tensor_copy(out=AT[p][64:128, qsl], in_=oB[0:HD, :])
                nc.vector.tensor_mul(AT[p][:, qsl], AT[p][:, qsl], R)

                if p == PAIRS - 1:
                    # output projection for q rows of chunk j (both pairs done)
                    for rt in range(QCH // 128):
                        rsl = slice(j * QCH + rt * 128, j * QCH + (rt + 1) * 128)
                        for nch in range(DIM // 512):
                            nsl = slice(nch * 512, (nch + 1) * 512)
                            po = psum_mm.tile([128, 512], f32, tag="mm")
                            for pp in range(PAIRS):
                                nc.tensor.matmul(
                                    po, AT[pp][:, rsl],
                                    ow_sb[:, pp, nsl],
                                    start=(pp == 0), stop=(pp == PAIRS - 1))
                            ot = outst_pool.tile([128, 512], f32, tag="ot")
                            nc.vector.tensor_copy(out=ot, in_=po)
                            nc.sync.dma_start(out=out[rsl, nsl], in_=ot)

        if debug:
            nc.sync.dma_start(out=dbg["dAT0"], in_=AT[0])

    nc.compile()
    return nc


def _get_nc():
    global _nc_cache
    if _nc_cache is None:
        _nc_cache = _build()
    return _nc_cache


def _prep_inputs(x, qw, kw, vw, ow):
    # undo interleaved head split: head h = cols h::16 -> contiguous blocks
    perm = np.concatenate([np.arange(h, DIM, HEADS) for h in range(HEADS)])
    qw_p = (qw[:, perm] / np.float32(np.sqrt(DIM))).astype(np.float32)
    kw_p = np.ascontiguousarray(kw[:, perm])
    vw_p = np.ascontiguousarray(vw[:, perm])
    ow_p = np.ascontiguousarray(ow[perm, :])

    kp = np.arange(128)[:, None]
    qf = np.arange(QCH)[None, :]
    masks = np.stack([(128 * i + kp <= qf) for i in range(4)]).astype(np.float32)

    in_maps = []
    for c in range(N_CORES):
        b, hg = c // 4, c % 4
        csl = slice(hg * 256, (hg + 1) * 256)
        in_maps.append({
            "xT": np.ascontiguousarray(x[b].T),
            "qw": np.ascontiguousarray(qw_p[:, csl]),
            "kw": np.ascontiguousarray(kw_p[:, csl]),
            "vw": np.ascontiguousarray(vw_p[:, csl]),
            "ow": np.ascontiguousarray(ow_p[csl, :]),
            "masks": masks,
        })
    return in_maps


def kernel(x, qw, kw, vw, ow, _trace=False):
    from concourse.bass_utils import run_bass_kernel_spmd

    if _trace:
        _install_ntff_hook()

    nc = _get_nc()
    in_maps = _prep_inputs(x, qw, kw, vw, ow)
    res = run_bass_kernel_spmd(nc, in_maps, core_ids=list(range(N_CORES)),
                               trace=_trace)
    parts = [r["out"] for r in res.results]
    outb = [parts[0] + parts[1] + parts[2] + parts[3],
            parts[4] + parts[5] + parts[6] + parts[7]]
    full = np.stack(outb).astype(np.float32)
    if _trace:
        kernel.last_results = res
        if res.exec_time_ns is not None:
            print(f"HW exec time: {res.exec_time_ns} ns")
        if res.instructions_and_trace:
            print(f"trace: {res.instructions_and_trace[1]}")
    return full


def _install_ntff_hook():
    """The image's antenv lacks axon_hooks; synthesize it so trace=True works."""
    import types
    if 'antenv.axon_hooks' in sys.modules:
        return
    mod = types.ModuleType('antenv.axon_hooks')
    mod._hook = None
    mod.set_axon_ntff_profile_hook = lambda h: setattr(mod, '_hook', h)
    mod.get_axon_ntff_profile_hook = lambda: mod._hook
    sys.modules['antenv.axon_hooks'] = mod
    import antenv
    antenv.axon_hooks = mod
    from trn_agent_boot.trn_boot import _ntff_profile_via_ctypes
    mod.set_axon_ntff_profile_hook(
        _ntff_profile_via_ctypes('/opt/axon/libaxon_pjrt.so'))


# revision 21
# speedup vs baseline: 1.0025x; 1.0025x over previous
"""Multi-head causal attention (B=2, S=2048, D=1024, H=16) on 8 TRN2 cores.

Sharding: core = (batch, group-of-4-heads). Each core computes attention for
its 4 heads of its batch and a rank-256 partial of the output projection;
the host sums the 4 partials per batch. The interleaved head split of the
reference (head h = columns h::16) is undone on the host by permuting the
weight matrices, so on-chip everything is head-contiguous.
"""
import sys
sys.path.insert(0, '/opt/trn_rl_repo')

import numpy as np

DIM = 1024
HEADS = 16
S = 2048
B = 2
HD = 64
N_CORES = 8
HPC = 4          # heads per core
PAIRS = 2        # processed as 2 pairs of heads (pair packs the 128-wide PE)
QCH = 512        # q chunk
NKT = S // 128   # k tiles per sequence

_nc_cache = None


def _build(debug=False):
    import concourse.tile as tile
    import concourse.mybir as mybir
    from concourse import bacc
    from concourse.masks import make_identity
    from contextlib import ExitStack

    f32 = mybir.dt.float32
    f32r = mybir.dt.float32r
    Exp = mybir.ActivationFunctionType.Exp

    nc = bacc.Bacc("TRN2", target_bir_lowering=False, debug=False,
                   enable_asserts=False, num_devices=N_CORES)

    xT = nc.dram_tensor("xT", [DIM, S], f32r, kind="ExternalInput").ap()
    qw = nc.dram_tensor("qw", [DIM, 256], f32r, kind="ExternalInput").ap()
    kw = nc.dram_tensor("kw", [DIM, 256], f32r, kind="ExternalInput").ap()
    vw = nc.dram_tensor("vw", [DIM, 256], f32r, kind="ExternalInput").ap()
    ow = nc.dram_tensor("ow", [256, DIM], f32r, kind="ExternalInput").ap()
    masks = nc.dram_tensor("masks", [4, 128, QCH], f32, kind="ExternalInput").ap()
    out = nc.dram_tensor("out", [S, DIM], f32, kind="ExternalOutput").ap()
    if debug:
        dbg = {}
        for name, shape, dt_ in (
                ("dQT0", [128, S], f32r), ("dKT0", [128, S], f32r),
                ("dVT0", [128, S], f32), ("dVaug0", [128, NKT, HD + 1], f32r),
                ("dAT0", [128, S], f32r), ("dP0", [128, QCH], f32r),
                ("dS0", [128, QCH], f32), ("dO0", [HD + 1, QCH], f32),
                ("dR0", [128, QCH], f32)):
            dbg[name] = nc.dram_tensor(name, shape, dt_, kind="ExternalOutput").ap()

    with tile.TileContext(nc) as tc, ExitStack() as ctx:
        const_pool = ctx.enter_context(tc.tile_pool(name="const", bufs=1))
        xin_pool = ctx.enter_context(tc.tile_pool(name="xin", bufs=2))
        big_pool = ctx.enter_context(tc.tile_pool(name="big", bufs=1))
        pt_pool = ctx.enter_context(tc.tile_pool(name="pt", bufs=6))
        small_pool = ctx.enter_context(tc.tile_pool(name="small", bufs=4))
        outst_pool = ctx.enter_context(tc.tile_pool(name="outst", bufs=4))
        psum_mm = ctx.enter_context(tc.tile_pool(name="psum_mm", bufs=4, space="PSUM"))
        psum_o = ctx.enter_context(tc.tile_pool(name="psum_o", bufs=4, space="PSUM"))

        # constants
        qw_sb = const_pool.tile([128, 8, 256], f32r, tag="qw")
        kw_sb = const_pool.tile([128, 8, 256], f32r, tag="kw")
        vw_sb = const_pool.tile([128, 8, 256], f32r, tag="vw")
        nc.sync.dma_start(out=qw_sb, in_=qw.rearrange("(kt p) m -> p kt m", p=128))
        nc.sync.dma_start(out=kw_sb, in_=kw.rearrange("(kt p) m -> p kt m", p=128))
        nc.sync.dma_start(out=vw_sb, in_=vw.rearrange("(kt p) m -> p kt m", p=128))
        ow_sb = const_pool.tile([128, 2, DIM], f32r, tag="ow")
        nc.sync.dma_start(out=ow_sb, in_=ow.rearrange("(t p) n -> p t n", p=128))
        masks_sb = const_pool.tile([128, 4, QCH], f32, tag="masks")
        nc.sync.dma_start(out=masks_sb, in_=masks.rearrange("i p q -> p i q"))
        ident = const_pool.tile([128, 128], f32, tag="ident")
        make_identity(nc, ident)

        QT = [big_pool.tile([128, S], f32r, tag=f"QT{p}", name=f"QT{p}") for p in range(PAIRS)]
        KT = [big_pool.tile([128, S], f32r, tag=f"KT{p}", name=f"KT{p}") for p in range(PAIRS)]
        VT = [big_pool.tile([128, S], f32, tag=f"VT{p}", name=f"VT{p}") for p in range(PAIRS)]
        AT = [big_pool.tile([128, S], f32r, tag=f"AT{p}", name=f"AT{p}") for p in range(PAIRS)]
        Vaug = [big_pool.tile([128, NKT, HD + 1], f32r, tag=f"Vaug{h}", name=f"Vaug{h}")
                for h in range(HPC)]
        ones_c = const_pool.tile([128, NKT, 1], f32, tag="ones")
        nc.vector.memset(ones_c, 1.0)
        for h in range(HPC):
            nc.vector.tensor_copy(out=Vaug[h][:, :, HD:HD + 1], in_=ones_c)
        Ef = const_pool.tile([33, 128], f32, tag="Ef")
        nc.vector.memset(Ef, 0.0)
        nc.vector.memset(Ef[0:1, 0:64], 1.0)
        nc.vector.memset(Ef[32:33, 64:128], 1.0)


        # ---- Phase 1: QKV projections (QT/KT/VT = W.T @ x.T, head-major) ----
        for j in range(S // QCH):
            qsl = slice(j * QCH, (j + 1) * QCH)
            xt = xin_pool.tile([128, 8, QCH], f32r, tag="xt")
            nc.sync.dma_start(
                out=xt,
                in_=xT.rearrange("(kt p) n -> p kt n", p=128)[:, :, qsl])
            for wsb, dstT in ((qw_sb, QT), (kw_sb, KT), (vw_sb, VT)):
                for ct in range(PAIRS):
                    ps = psum_mm.tile([128, QCH], f32, tag="mm")
                    for kt in range(8):
                        nc.tensor.matmul(
                            ps,
                            wsb[:, kt, ct * 128:(ct + 1) * 128],
                            xt[:, kt, :],
                            start=(kt == 0), stop=(kt == 7))
                    nc.vector.tensor_copy(out=dstT[ct][:, qsl], in_=ps)

        if debug:
            nc.sync.dma_start(out=dbg["dQT0"], in_=QT[0])
            nc.sync.dma_start(out=dbg["dKT0"], in_=KT[0])
            nc.sync.dma_start(out=dbg["dVT0"], in_=VT[0])

        # ---- Phase 1b: V -> Vaug (transpose to [k, hd] + ones column) ----
        for p in range(PAIRS):
            for kt in range(NKT):
                pst = psum_mm.tile([128, 128], f32, tag="mm")
                nc.tensor.transpose(pst, VT[p][:, kt * 128:(kt + 1) * 128], ident)
                for hh in range(2):
                    nc.vector.tensor_copy(
                        out=Vaug[2 * p + hh][:, kt, 0:HD],
                        in_=pst[:, hh * 64:(hh + 1) * 64])

        if debug:
            nc.sync.dma_start(out=dbg["dVaug0"], in_=Vaug[0])

        # ---- Phase 2: causal attention, pair-packed; then output proj ----
        for p in range(PAIRS):
            hA, hB = 2 * p, 2 * p + 1
            for j in range(S // QCH):
                nkt = 4 * (j + 1)
                qsl = slice(j * QCH, (j + 1) * QCH)
                oA = psum_o.tile([HD + 1, QCH], f32, tag="o")
                oB = psum_o.tile([HD + 1, QCH], f32, tag="o")
                for kt in range(nkt):
                    ksl = slice(kt * 128, (kt + 1) * 128)
                    sA = psum_mm.tile([128, QCH], f32, tag="mm")
                    sB = psum_mm.tile([128, QCH], f32, tag="mm")
                    nc.tensor.matmul(sA, KT[p][0:64, ksl],
                                     QT[p][0:64, qsl],
                                     start=True, stop=True)
                    nc.tensor.matmul(sB, KT[p][64:128, ksl],
                                     QT[p][64:128, qsl],
                                     start=True, stop=True)
                    pA = pt_pool.tile([128, QCH], f32r, tag="pt")
                    pB = pt_pool.tile([128, QCH], f32r, tag="pt")
                    nc.scalar.activation(out=pA, in_=sA, func=Exp)
                    nc.scalar.activation(out=pB, in_=sB, func=Exp)
                    if debug and p == 0 and j == 0 and kt == 0:
                        sstage = small_pool.tile([128, QCH], f32, tag="dbgs")
                        nc.vector.tensor_copy(out=sstage, in_=sA)
                        nc.sync.dma_start(out=dbg["dS0"], in_=sstage)
                    di = kt - 4 * j
                    if di >= 0:  # diagonal tile: apply causal mask
                        nc.vector.tensor_mul(pA, pA, masks_sb[:, di, :])
                        nc.vector.tensor_mul(pB, pB, masks_sb[:, di, :])
                    if debug and p == 0 and j == 0 and kt == 0:
                        nc.sync.dma_start(out=dbg["dP0"], in_=pA)
                    nc.tensor.matmul(oA, Vaug[hA][:, kt, :],
                                     pA,
                                     start=(kt == 0), stop=(kt == nkt - 1))
                    nc.tensor.matmul(oB, Vaug[hB][:, kt, :],
                                     pB,
                                     start=(kt == 0), stop=(kt == nkt - 1))
                # normalize: A = O / l  (l = ones-row of oX), store transposed
                rAB = small_pool.tile([33, QCH], f32, tag="r")
                nc.vector.memset(rAB, 0.0)
                with nc.allow_low_precision(reason="recip rows"):
                    nc.vector.reciprocal(rAB[0:1, :], oA[HD:HD + 1, :])
                    nc.vector.reciprocal(rAB[32:33, :], oB[HD:HD + 1, :])
                R = psum_mm.tile([128, QCH], f32, tag="mm")
                nc.tensor.matmul(R, Ef, rAB, start=True, stop=True)
                if debug and p == 0 and j == 0:
                    ostage = small_pool.tile([HD + 1, QCH], f32, tag="dbgo")
                    nc.vector.tensor_copy(out=ostage, in_=oA)
                    nc.sync.dma_start(out=dbg["dO0"], in_=ostage)
                    rstage = small_pool.tile([128, QCH], f32, tag="dbgR")
                    nc.vector.tensor_copy(out=rstage, in_=R)
                    nc.sync.dma_start(out=dbg["dR0"], in_=rstage)
                nc.vector.tensor_copy(out=AT[p][0:64, qsl], in_=oA[0:HD, :])
                nc.vector.tensor_copy(out=AT[p][64:128, qsl], in_=oB[0:HD, :])
                nc.vector.tensor_mul(AT[p][:, qsl], AT[p][:, qsl], R)

                if p == PAIRS - 1:
                    # output projection for q rows of chunk j (both pairs done)
                    for rt in range(QCH // 128):
                        rsl = slice(j * QCH + rt * 128, j * QCH + (rt + 1) * 128)
                        for nch in range(DIM // 512):
                            nsl = slice(nch * 512, (nch + 1) * 512)
                            po = psum_mm.tile([128, 512], f32, tag="mm")
                            for pp in range(PAIRS):
                                nc.tensor.matmul(
                                    po, AT[pp][:, rsl],
                                    ow_sb[:, pp, nsl],
                                    start=(pp == 0), stop=(pp == PAIRS - 1))
                            ot = outst_pool.tile([128, 512], f32, tag="ot")
                            nc.vector.tensor_copy(out=ot, in_=po)
                            nc.sync.dma_start(out=out[rsl, nsl], in_=ot)

        if debug:
            nc.sync.dma_start(out=dbg["dAT0"], in_=AT[0])

    nc.compile()
    return nc


def _get_nc():
    global _nc_cache
    if _nc_cache is None:
        _nc_cache = _build()
    return _nc_cache


def _prep_inputs(x, qw, kw, vw, ow):
    # undo interleaved head split: head h = cols h::16 -> contiguous blocks
    perm = np.concatenate([np.arange(h, DIM, HEADS) for h in range(HEADS)])
    qw_p = (qw[:, perm] / np.float32(np.sqrt(DIM))).astype(np.float32)
    kw_p = np.ascontiguousarray(kw[:, perm])
    vw_p = np.ascontiguousarray(vw[:, perm])
    ow_p = np.ascontiguousarray(ow[perm, :])

    kp = np.arange(128)[:, None]
    qf = np.arange(QCH)[None, :]
    masks = np.stack([(128 * i + kp <= qf) for i in range(4)]).astype(np.float32)

    in_maps = []
    for c in range(N_CORES):
        b, hg = c // 4, c % 4
        csl = slice(hg * 256, (hg + 1) * 256)
        in_maps.append({
            "xT": np.ascontiguousarray(x[b].T),
            "qw": np.ascontiguousarray(qw_p[:, csl]),
            "kw": np.ascontiguousarray(kw_p[:, csl]),
            "vw": np.ascontiguousarray(vw_p[:, csl]),
            "ow": np.ascontiguousarray(ow_p[csl, :]),
            "masks": masks,
        })
    return in_maps


def kernel(x, qw, kw, vw, ow, _trace=False):
    from concourse.bass_utils import run_bass_kernel_spmd

    if _trace:
        _install_ntff_hook()

    nc = _get_nc()
    in_maps = _prep_inputs(x, qw, kw, vw, ow)
    res = run_bass_kernel_spmd(nc, in_maps, core_ids=list(range(N_CORES)),
                               trace=_trace)
    parts = [r["out"] for r in res.results]
    outb = [parts[0] + parts[1] + parts[2] + parts[3],
            parts[4] + parts[5] + parts[6] + parts[7]]
    full = np.stack(outb).astype(np.float32)
    if _trace:
        kernel.last_results = res
        if res.exec_time_ns is not None:
            print(f"HW exec time: {res.exec_time_ns} ns")
        if res.instructions_and_trace:
            print(f"trace: {res.instructions_and_trace[1]}")
    return full


def _install_ntff_hook():
    """The image's antenv lacks axon_hooks; synthesize it so trace=True works."""
    import types
    if 'antenv.axon_hooks' in sys.modules:
        return
    mod = types.ModuleType('antenv.axon_hooks')
    mod._hook = None
    mod.set_axon_ntff_profile_hook = lambda h: setattr(mod, '_hook', h)
    mod.get_axon_ntff_profile_hook = lambda: mod._hook
    sys.modules['antenv.axon_hooks'] = mod
    import antenv
    antenv.axon_hooks = mod
    from trn_agent_boot.trn_boot import _ntff_profile_via_ctypes
    mod.set_axon_ntff_profile_hook(
        _ntff_profile_via_ctypes('/opt/axon/libaxon_pjrt.so'))


# revision 22
# speedup vs baseline: 1.0238x; 1.0213x over previous
"""Multi-head causal attention (B=2, S=2048, D=1024, H=16) on 8 TRN2 cores.

Sharding: core = (batch, group-of-4-heads). Each core computes attention for
its 4 heads of its batch and a rank-256 partial of the output projection;
the host sums the 4 partials per batch. The interleaved head split of the
reference (head h = columns h::16) is undone on the host by permuting the
weight matrices, so on-chip everything is head-contiguous.
"""
import sys
sys.path.insert(0, '/opt/trn_rl_repo')

import numpy as np

DIM = 1024
HEADS = 16
S = 2048
B = 2
HD = 64
N_CORES = 8
HPC = 4          # heads per core
PAIRS = 2        # processed as 2 pairs of heads (pair packs the 128-wide PE)
QCH = 512        # q chunk
NKT = S // 128   # k tiles per sequence

_nc_cache = None


def _build(debug=False):
    import concourse.tile as tile
    import concourse.mybir as mybir
    from concourse import bacc
    from concourse.masks import make_identity
    from contextlib import ExitStack

    f32 = mybir.dt.float32
    f32r = mybir.dt.float32r
    Exp = mybir.ActivationFunctionType.Exp

    nc = bacc.Bacc("TRN2", target_bir_lowering=False, debug=False,
                   enable_asserts=False, num_devices=N_CORES)

    xT = nc.dram_tensor("xT", [DIM, S], f32r, kind="ExternalInput").ap()
    qw = nc.dram_tensor("qw", [DIM, 256], f32r, kind="ExternalInput").ap()
    kw = nc.dram_tensor("kw", [DIM, 256], f32r, kind="ExternalInput").ap()
    vw = nc.dram_tensor("vw", [DIM, 256], f32r, kind="ExternalInput").ap()
    ow = nc.dram_tensor("ow", [256, DIM], f32r, kind="ExternalInput").ap()
    masks = nc.dram_tensor("masks", [4, 128, QCH], f32, kind="ExternalInput").ap()
    out = nc.dram_tensor("out", [S, DIM], f32, kind="ExternalOutput").ap()
    if debug:
        dbg = {}
        for name, shape, dt_ in (
                ("dQT0", [128, S], f32r), ("dKT0", [128, S], f32r),
                ("dVT0", [128, S], f32), ("dVaug0", [128, NKT, HD + 1], f32r),
                ("dAT0", [128, S], f32r), ("dP0", [128, QCH], f32r),
                ("dS0", [128, QCH], f32), ("dO0", [HD + 1, QCH], f32),
                ("dR0", [128, QCH], f32)):
            dbg[name] = nc.dram_tensor(name, shape, dt_, kind="ExternalOutput").ap()

    with tile.TileContext(nc) as tc, ExitStack() as ctx:
        const_pool = ctx.enter_context(tc.tile_pool(name="const", bufs=1))
        xin_pool = ctx.enter_context(tc.tile_pool(name="xin", bufs=2))
        big_pool = ctx.enter_context(tc.tile_pool(name="big", bufs=1))
        pt_pool = ctx.enter_context(tc.tile_pool(name="pt", bufs=6))
        small_pool = ctx.enter_context(tc.tile_pool(name="small", bufs=4))
        outst_pool = ctx.enter_context(tc.tile_pool(name="outst", bufs=4))
        psum_mm = ctx.enter_context(tc.tile_pool(name="psum_mm", bufs=4, space="PSUM"))
        psum_o = ctx.enter_context(tc.tile_pool(name="psum_o", bufs=4, space="PSUM"))

        # constants
        qw_sb = const_pool.tile([128, 8, 256], f32r, tag="qw")
        kw_sb = const_pool.tile([128, 8, 256], f32r, tag="kw")
        vw_sb = const_pool.tile([128, 8, 256], f32r, tag="vw")
        nc.sync.dma_start(out=qw_sb, in_=qw.rearrange("(kt p) m -> p kt m", p=128))
        nc.sync.dma_start(out=kw_sb, in_=kw.rearrange("(kt p) m -> p kt m", p=128))
        nc.sync.dma_start(out=vw_sb, in_=vw.rearrange("(kt p) m -> p kt m", p=128))
        ow_sb = const_pool.tile([128, 2, DIM], f32r, tag="ow")
        nc.sync.dma_start(out=ow_sb, in_=ow.rearrange("(t p) n -> p t n", p=128))
        masks_sb = const_pool.tile([128, 4, QCH], f32, tag="masks")
        nc.sync.dma_start(out=masks_sb, in_=masks.rearrange("i p q -> p i q"))
        ident = const_pool.tile([128, 128], f32, tag="ident")
        make_identity(nc, ident)

        QT = [big_pool.tile([128, S], f32r, tag=f"QT{p}", name=f"QT{p}") for p in range(PAIRS)]
        KT = [big_pool.tile([128, S], f32r, tag=f"KT{p}", name=f"KT{p}") for p in range(PAIRS)]
        VT = [big_pool.tile([128, S], f32, tag=f"VT{p}", name=f"VT{p}") for p in range(PAIRS)]
        AT = [big_pool.tile([128, S], f32r, tag=f"AT{p}", name=f"AT{p}") for p in range(PAIRS)]
        Vaug = [big_pool.tile([128, NKT, HD + 1], f32r, tag=f"Vaug{h}", name=f"Vaug{h}")
                for h in range(HPC)]
        ones_c = const_pool.tile([128, NKT, 1], f32, tag="ones")
        nc.vector.memset(ones_c, 1.0)
        for h in range(HPC):
            nc.vector.tensor_copy(out=Vaug[h][:, :, HD:HD + 1], in_=ones_c)
        Ef = const_pool.tile([33, 128], f32, tag="Ef")
        nc.vector.memset(Ef, 0.0)
        nc.vector.memset(Ef[0:1, 0:64], 1.0)
        nc.vector.memset(Ef[32:33, 64:128], 1.0)


        # ---- Phase 1: QKV projections (QT/KT/VT = W.T @ x.T, head-major) ----
        for j in range(S // QCH):
            qsl = slice(j * QCH, (j + 1) * QCH)
            xt = xin_pool.tile([128, 8, QCH], f32r, tag="xt")
            nc.sync.dma_start(
                out=xt,
                in_=xT.rearrange("(kt p) n -> p kt n", p=128)[:, :, qsl])
            for wsb, dstT in ((qw_sb, QT), (kw_sb, KT), (vw_sb, VT)):
                for ct in range(PAIRS):
                    ps = psum_mm.tile([128, QCH], f32, tag="mm")
                    for kt in range(8):
                        nc.tensor.matmul(
                            ps,
                            wsb[:, kt, ct * 128:(ct + 1) * 128],
                            xt[:, kt, :],
                            start=(kt == 0), stop=(kt == 7))
                    nc.vector.tensor_copy(out=dstT[ct][:, qsl], in_=ps)

        if debug:
            nc.sync.dma_start(out=dbg["dQT0"], in_=QT[0])
            nc.sync.dma_start(out=dbg["dKT0"], in_=KT[0])
            nc.sync.dma_start(out=dbg["dVT0"], in_=VT[0])

        # ---- Phase 1b: V -> Vaug (transpose to [k, hd] + ones column) ----
        for p in range(PAIRS):
            for kt in range(NKT):
                pst = psum_mm.tile([128, 128], f32, tag="mm")
                nc.tensor.transpose(pst, VT[p][:, kt * 128:(kt + 1) * 128], ident)
                for hh in range(2):
                    nc.vector.tensor_copy(
                        out=Vaug[2 * p + hh][:, kt, 0:HD],
                        in_=pst[:, hh * 64:(hh + 1) * 64])

        if debug:
            nc.sync.dma_start(out=dbg["dVaug0"], in_=Vaug[0])

        # ---- Phase 2: causal attention, pair-packed; then output proj ----
        # Finalize (recip + normalize + out-proj) for chunk i is EMITTED after
        # chunk i+1's matmuls so the slow DVE reciprocal never head-of-line
        # blocks the PE instruction stream (Tile schedules per-engine in
        # emission order).
        def finalize(p, j, oA, oB):
            qsl = slice(j * QCH, (j + 1) * QCH)
            rAB = small_pool.tile([33, QCH], f32, tag="r", name="rAB")
            nc.vector.memset(rAB, 0.0)
            with nc.allow_low_precision(reason="recip rows"):
                nc.vector.reciprocal(rAB[0:1, :], oA[HD:HD + 1, :])
                nc.vector.reciprocal(rAB[32:33, :], oB[HD:HD + 1, :])
            R = psum_mm.tile([128, QCH], f32, tag="mm", name="R")
            nc.tensor.matmul(R, Ef, rAB, start=True, stop=True)
            nc.vector.tensor_copy(out=AT[p][0:64, qsl], in_=oA[0:HD, :])
            nc.vector.tensor_copy(out=AT[p][64:128, qsl], in_=oB[0:HD, :])
            nc.vector.tensor_mul(AT[p][:, qsl], AT[p][:, qsl], R)

            if p == PAIRS - 1:
                # output projection for q rows of chunk j (both pairs done)
                for rt in range(QCH // 128):
                    rsl = slice(j * QCH + rt * 128, j * QCH + (rt + 1) * 128)
                    for nch in range(DIM // 512):
                        nsl = slice(nch * 512, (nch + 1) * 512)
                        po = psum_mm.tile([128, 512], f32, tag="mm", name="po")
                        for pp in range(PAIRS):
                            nc.tensor.matmul(
                                po, AT[pp][:, rsl],
                                ow_sb[:, pp, nsl],
                                start=(pp == 0), stop=(pp == PAIRS - 1))
                        ot = outst_pool.tile([128, 512], f32, tag="ot", name="ot")
                        nc.vector.tensor_copy(out=ot, in_=po)
                        nc.sync.dma_start(out=out[rsl, nsl], in_=ot)

        pending = None
        for p in range(PAIRS):
            hA, hB = 2 * p, 2 * p + 1
            for j in range(S // QCH):
                nkt = 4 * (j + 1)
                qsl = slice(j * QCH, (j + 1) * QCH)
                oA = psum_o.tile([HD + 1, QCH], f32, tag="o", name="oA")
                oB = psum_o.tile([HD + 1, QCH], f32, tag="o", name="oB")
                for kt in range(nkt):
                    ksl = slice(kt * 128, (kt + 1) * 128)
                    sA = psum_mm.tile([128, QCH], f32, tag="mm", name="sA")
                    sB = psum_mm.tile([128, QCH], f32, tag="mm", name="sB")
                    nc.tensor.matmul(sA, KT[p][0:64, ksl],
                                     QT[p][0:64, qsl],
                                     start=True, stop=True)
                    nc.tensor.matmul(sB, KT[p][64:128, ksl],
                                     QT[p][64:128, qsl],
                                     start=True, stop=True)
                    pA = pt_pool.tile([128, QCH], f32r, tag="pt", name="pA")
                    pB = pt_pool.tile([128, QCH], f32r, tag="pt", name="pB")
                    nc.scalar.activation(out=pA, in_=sA, func=Exp)
                    nc.scalar.activation(out=pB, in_=sB, func=Exp)
                    di = kt - 4 * j
                    if di >= 0:  # diagonal tile: apply causal mask
                        nc.vector.tensor_mul(pA, pA, masks_sb[:, di, :])
                        nc.vector.tensor_mul(pB, pB, masks_sb[:, di, :])
                    nc.tensor.matmul(oA, Vaug[hA][:, kt, :],
                                     pA,
                                     start=(kt == 0), stop=(kt == nkt - 1))
                    nc.tensor.matmul(oB, Vaug[hB][:, kt, :],
                                     pB,
                                     start=(kt == 0), stop=(kt == nkt - 1))
                    if pending is not None and kt == min(3, nkt - 1):
                        # chunk underway; safe to emit previous chunk's tail
                        finalize(*pending)
                        pending = None
                pending = (p, j, oA, oB)
        finalize(*pending)

        if debug:
            nc.sync.dma_start(out=dbg["dAT0"], in_=AT[0])

    nc.compile()
    return nc


def _get_nc():
    global _nc_cache
    if _nc_cache is None:
        _nc_cache = _build()
    return _nc_cache


def _prep_inputs(x, qw, kw, vw, ow):
    # undo interleaved head split: head h = cols h::16 -> contiguous blocks
    perm = np.concatenate([np.arange(h, DIM, HEADS) for h in range(HEADS)])
    qw_p = (qw[:, perm] / np.float32(np.sqrt(DIM))).astype(np.float32)
    kw_p = np.ascontiguousarray(kw[:, perm])
    vw_p = np.ascontiguousarray(vw[:, perm])
    ow_p = np.ascontiguousarray(ow[perm, :])

    kp = np.arange(128)[:, None]
    qf = np.arange(QCH)[None, :]
    masks = np.stack([(128 * i + kp <= qf) for i in range(4)]).astype(np.float32)

    in_maps = []
    for c in range(N_CORES):
        b, hg = c // 4, c % 4
        csl = slice(hg * 256, (hg + 1) * 256)
        in_maps.append({
            "xT": np.ascontiguousarray(x[b].T),
            "qw": np.ascontiguousarray(qw_p[:, csl]),
            "kw": np.ascontiguousarray(kw_p[:, csl]),
            "vw": np.ascontiguousarray(vw_p[:, csl]),
            "ow": np.ascontiguousarray(ow_p[csl, :]),
            "masks": masks,
        })
    return in_maps


def kernel(x, qw, kw, vw, ow, _trace=False):
    from concourse.bass_utils import run_bass_kernel_spmd

    if _trace:
        _install_ntff_hook()

    nc = _get_nc()
    in_maps = _prep_inputs(x, qw, kw, vw, ow)
    res = run_bass_kernel_spmd(nc, in_maps, core_ids=list(range(N_CORES)),
                               trace=_trace)
    parts = [r["out"] for r in res.results]
    outb = [parts[0] + parts[1] + parts[2] + parts[3],
            parts[4] + parts[5] + parts[6] + parts[7]]
    full = np.stack(outb).astype(np.float32)
    if _trace:
        kernel.last_results = res
        if res.exec_time_ns is not None:
            print(f"HW exec time: {res.exec_time_ns} ns")
        if res.instructions_and_trace:
            print(f"trace: {res.instructions_and_trace[1]}")
    return full


def _install_ntff_hook():
    """The image's antenv lacks axon_hooks; synthesize it so trace=True works."""
    import types
    if 'antenv.axon_hooks' in sys.modules:
        return
    mod = types.ModuleType('antenv.axon_hooks')
    mod._hook = None
    mod.set_axon_ntff_profile_hook = lambda h: setattr(mod, '_hook', h)
    mod.get_axon_ntff_profile_hook = lambda: mod._hook
    sys.modules['antenv.axon_hooks'] = mod
    import antenv
    antenv.axon_hooks = mod
    from trn_agent_boot.trn_boot import _ntff_profile_via_ctypes
    mod.set_axon_ntff_profile_hook(
        _ntff_profile_via_ctypes('/opt/axon/libaxon_pjrt.so'))


# revision 23
# speedup vs baseline: 1.0995x; 1.0739x over previous
"""Multi-head causal attention (B=2, S=2048, D=1024, H=16) on 8 TRN2 cores.

Sharding: core = (batch, group-of-4-heads). Each core computes attention for
its 4 heads of its batch and a rank-256 partial of the output projection;
the host sums the 4 partials per batch. The interleaved head split of the
reference (head h = columns h::16) is undone on the host by permuting the
weight matrices, so on-chip everything is head-contiguous.

On-chip layout (per core, all matmuls in fp32r):
  QT/KT [128, 2048]   head-pair-stacked transposed Q/K (pair p, heads A/B on
                      partitions 0:64 / 64:128)
  S^T   [128, 2, 512] scores for a (k-tile, q-chunk), both heads, one 2-bank
                      PSUM tile; K=64 matmuls row-packed in the PE array
  exp   one ScalarE activation per k-tile over both heads' scores
  PV    oAB[65, 2, 512] += Vaug^T @ P^T; Vaug carries a ones column so row 64
                      accumulates the softmax denominator l
  norm  1/l broadcast via DRAM-roundtrip DMA (stride-0 partition reads are
                      only legal from DRAM), A^T scaled on VectorE
  out   partial = A^T.T @ ow, accumulated over the 2 pairs in PSUM
"""
import sys
sys.path.insert(0, '/opt/trn_rl_repo')

import numpy as np

DIM = 1024
HEADS = 16
S = 2048
B = 2
HD = 64
N_CORES = 8
HPC = 4          # heads per core
PAIRS = 2        # processed as 2 pairs of heads (pair packs the 128-wide PE)
QCH = 512        # q chunk
NKT = S // 128   # k tiles per sequence

_nc_cache = None


def _build(debug=False):
    import concourse.bass as bass
    import concourse.tile as tile
    import concourse.mybir as mybir
    from concourse import bacc
    from concourse.masks import make_identity
    from contextlib import ExitStack

    f32 = mybir.dt.float32
    f32r = mybir.dt.float32r
    Exp = mybir.ActivationFunctionType.Exp

    def bc(ap, n):
        # stride-0 partition broadcast of a [1, ...] DRAM AP to n partitions
        return bass.AP(tensor=ap.tensor, offset=ap.offset,
                       ap=[[0, n]] + [list(d) for d in ap.ap[1:]])

    nc = bacc.Bacc("TRN2", target_bir_lowering=False, debug=False,
                   enable_asserts=False, num_devices=N_CORES)

    xT = nc.dram_tensor("xT", [DIM, S], f32r, kind="ExternalInput").ap()
    qw = nc.dram_tensor("qw", [DIM, 256], f32r, kind="ExternalInput").ap()
    kw = nc.dram_tensor("kw", [DIM, 256], f32r, kind="ExternalInput").ap()
    vw = nc.dram_tensor("vw", [DIM, 256], f32r, kind="ExternalInput").ap()
    ow = nc.dram_tensor("ow", [256, DIM], f32r, kind="ExternalInput").ap()
    masks = nc.dram_tensor("masks", [4, 128, QCH], f32, kind="ExternalInput").ap()
    out = nc.dram_tensor("out", [S, DIM], f32, kind="ExternalOutput").ap()
    if debug:
        dbg = {}
        for name, shape, dt_ in (
                ("dQT0", [128, S], f32r), ("dKT0", [128, S], f32r),
                ("dVT0", [128, S], f32), ("dVaug0", [128, NKT, HD + 1], f32r),
                ("dAT0", [128, S], f32r)):
            dbg[name] = nc.dram_tensor(name, shape, dt_, kind="ExternalOutput").ap()

    with tile.TileContext(nc) as tc, ExitStack() as ctx:
        const_pool = ctx.enter_context(tc.tile_pool(name="const", bufs=1))
        xin_pool = ctx.enter_context(tc.tile_pool(name="xin", bufs=2))
        big_pool = ctx.enter_context(tc.tile_pool(name="big", bufs=1))
        pt_pool = ctx.enter_context(tc.tile_pool(name="pt", bufs=4))
        small_pool = ctx.enter_context(tc.tile_pool(name="small", bufs=4))
        outst_pool = ctx.enter_context(tc.tile_pool(name="outst", bufs=3))
        dram_pool = ctx.enter_context(tc.tile_pool(name="dram", bufs=2, space="DRAM"))
        psum_s = ctx.enter_context(tc.tile_pool(name="psum_s", bufs=2, space="PSUM"))
        psum_o = ctx.enter_context(tc.tile_pool(name="psum_o", bufs=2, space="PSUM"))

        # constants
        qw_sb = const_pool.tile([128, 8, 256], f32r, tag="qw")
        kw_sb = const_pool.tile([128, 8, 256], f32r, tag="kw")
        vw_sb = const_pool.tile([128, 8, 256], f32r, tag="vw")
        nc.sync.dma_start(out=qw_sb, in_=qw.rearrange("(kt p) m -> p kt m", p=128))
        nc.sync.dma_start(out=kw_sb, in_=kw.rearrange("(kt p) m -> p kt m", p=128))
        nc.sync.dma_start(out=vw_sb, in_=vw.rearrange("(kt p) m -> p kt m", p=128))
        ow_sb = const_pool.tile([128, 2, DIM], f32r, tag="ow")
        nc.sync.dma_start(out=ow_sb, in_=ow.rearrange("(t p) n -> p t n", p=128))
        masks_sb = const_pool.tile([128, 4, QCH], f32, tag="masks")
        nc.sync.dma_start(out=masks_sb, in_=masks.rearrange("i p q -> p i q"))
        ident = const_pool.tile([128, 128], f32, tag="ident")
        make_identity(nc, ident)

        QT = [big_pool.tile([128, S], f32r, tag=f"QT{p}", name=f"QT{p}")
              for p in range(PAIRS)]
        KT = [big_pool.tile([128, S], f32r, tag=f"KT{p}", name=f"KT{p}")
              for p in range(PAIRS)]
        VT = [big_pool.tile([128, S], f32, tag=f"VT{p}", name=f"VT{p}")
              for p in range(PAIRS)]
        AT = [big_pool.tile([128, S], f32r, tag=f"AT{p}", name=f"AT{p}")
              for p in range(PAIRS)]
        Vaug = [big_pool.tile([128, NKT, HD + 1], f32r, tag=f"Vaug{h}",
                              name=f"Vaug{h}") for h in range(HPC)]
        ones_c = const_pool.tile([128, NKT, 1], f32, tag="ones")
        nc.vector.memset(ones_c, 1.0)
        for h in range(HPC):
            nc.vector.tensor_copy(out=Vaug[h][:, :, HD:HD + 1], in_=ones_c)

        # ---- Phase 1: QKV projections (QT/KT/VT = W.T @ x.T, head-major) ----
        for j in range(S // QCH):
            qsl = slice(j * QCH, (j + 1) * QCH)
            xt = xin_pool.tile([128, 8, QCH], f32r, tag="xt")
            nc.sync.dma_start(
                out=xt,
                in_=xT.rearrange("(kt p) n -> p kt n", p=128)[:, :, qsl])
            for wsb, dstT in ((qw_sb, QT), (kw_sb, KT), (vw_sb, VT)):
                ps2 = psum_s.tile([128, 2, QCH], f32, tag="s2", name="ps2")
                for ct in range(PAIRS):
                    for kt in range(8):
                        nc.tensor.matmul(
                            ps2[:, ct, :],
                            wsb[:, kt, ct * 128:(ct + 1) * 128],
                            xt[:, kt, :],
                            start=(kt == 0), stop=(kt == 7))
                for ct in range(PAIRS):
                    nc.vector.tensor_copy(out=dstT[ct][:, qsl], in_=ps2[:, ct, :])

        if debug:
            nc.sync.dma_start(out=dbg["dQT0"], in_=QT[0])
            nc.sync.dma_start(out=dbg["dKT0"], in_=KT[0])
            nc.sync.dma_start(out=dbg["dVT0"], in_=VT[0])

        # ---- Phase 1b: V -> Vaug (transpose to [k, hd] + ones column) ----
        for p in range(PAIRS):
            for kt in range(NKT):
                pst = psum_s.tile([128, 128], f32, tag="s2", name="pst")
                nc.tensor.transpose(pst, VT[p][:, kt * 128:(kt + 1) * 128], ident)
                for hh in range(2):
                    nc.vector.tensor_copy(
                        out=Vaug[2 * p + hh][:, kt, 0:HD],
                        in_=pst[:, hh * 64:(hh + 1) * 64])

        if debug:
            nc.sync.dma_start(out=dbg["dVaug0"], in_=Vaug[0])

        # ---- Phase 2: causal attention, pair-packed; then output proj ----
        # Finalize (recip + normalize + out-proj) for chunk i is EMITTED after
        # chunk i+1's first matmuls so the slow DVE reciprocal never
        # head-of-line blocks the PE instruction stream.
        def finalize(p, j, oAB):
            qsl = slice(j * QCH, (j + 1) * QCH)
            rAB = small_pool.tile([33, QCH], f32, tag="r", name="rAB")
            nc.vector.memset(rAB, 0.0)
            with nc.allow_low_precision(reason="recip rows"):
                nc.vector.reciprocal(rAB[0:1, :], oAB[HD:HD + 1, 0, :])
                nc.vector.reciprocal(rAB[32:33, :], oAB[HD:HD + 1, 1, :])
            rd = dram_pool.tile([33, QCH], f32, tag="rd", name="rd")
            nc.sync.dma_start(out=rd, in_=rAB)
            Rsb = small_pool.tile([128, QCH], f32, tag="Rsb", name="Rsb")
            nc.sync.dma_start(out=Rsb[0:64, :], in_=bc(rd[0:1, :], 64))
            nc.sync.dma_start(out=Rsb[64:128, :], in_=bc(rd[32:33, :], 64))
            nc.vector.tensor_copy(out=AT[p][0:64, qsl], in_=oAB[0:HD, 0, :])
            nc.vector.tensor_copy(out=AT[p][64:128, qsl], in_=oAB[0:HD, 1, :])
            nc.vector.tensor_mul(AT[p][:, qsl], AT[p][:, qsl], Rsb)

            if p == PAIRS - 1:
                # output projection for q rows of chunk j (both pairs done)
                for rt in range(QCH // 128):
                    rsl = slice(j * QCH + rt * 128, j * QCH + (rt + 1) * 128)
                    po2 = psum_s.tile([128, 2, 512], f32, tag="s2", name="po2")
                    for nch in range(DIM // 512):
                        for pp in range(PAIRS):
                            nc.tensor.matmul(
                                po2[:, nch, :], AT[pp][:, rsl],
                                ow_sb[:, pp, nch * 512:(nch + 1) * 512],
                                start=(pp == 0), stop=(pp == PAIRS - 1))
                    ot = outst_pool.tile([128, 2, 512], f32, tag="ot", name="ot")
                    nc.vector.tensor_copy(out=ot, in_=po2)
                    nc.sync.dma_start(
                        out=out[rsl, :].rearrange("p (a b) -> p a b", a=2),
                        in_=ot)

        pending = None
        for p in range(PAIRS):
            hA, hB = 2 * p, 2 * p + 1
            for j in range(S // QCH):
                nkt = 4 * (j + 1)
                qsl = slice(j * QCH, (j + 1) * QCH)
                oAB = psum_o.tile([HD + 1, 2, QCH], f32, tag="o", name="oAB")
                for kt in range(nkt):
                    ksl = slice(kt * 128, (kt + 1) * 128)
                    sAB = psum_s.tile([128, 2, QCH], f32, tag="s2", name="sAB")
                    for hh in range(2):
                        nc.tensor.matmul(sAB[:, hh, :],
                                         KT[p][hh * 64:(hh + 1) * 64, ksl],
                                         QT[p][hh * 64:(hh + 1) * 64, qsl],
                                         start=True, stop=True)
                    pAB = pt_pool.tile([128, 2, QCH], f32r, tag="pt", name="pAB")
                    nc.scalar.activation(out=pAB, in_=sAB, func=Exp)
                    di = kt - 4 * j
                    if di >= 0:  # diagonal tile: apply causal mask
                        for hh in range(2):
                            nc.vector.tensor_mul(pAB[:, hh, :], pAB[:, hh, :],
                                                 masks_sb[:, di, :])
                    for hh in range(2):
                        nc.tensor.matmul(oAB[:, hh, :],
                                         Vaug[2 * p + hh][:, kt, :],
                                         pAB[:, hh, :],
                                         start=(kt == 0), stop=(kt == nkt - 1))
                    if pending is not None and kt == min(3, nkt - 1):
                        # chunk underway; safe to emit previous chunk's tail
                        finalize(*pending)
                        pending = None
                pending = (p, j, oAB)
        finalize(*pending)

        if debug:
            nc.sync.dma_start(out=dbg["dAT0"], in_=AT[0])

    nc.compile()
    return nc


def _get_nc():
    global _nc_cache
    if _nc_cache is None:
        _nc_cache = _build()
    return _nc_cache


def _prep_inputs(x, qw, kw, vw, ow):
    # undo interleaved head split: head h = cols h::16 -> contiguous blocks
    perm = np.concatenate([np.arange(h, DIM, HEADS) for h in range(HEADS)])
    qw_p = (qw[:, perm] / np.float32(np.sqrt(DIM))).astype(np.float32)
    kw_p = np.ascontiguousarray(kw[:, perm])
    vw_p = np.ascontiguousarray(vw[:, perm])
    ow_p = np.ascontiguousarray(ow[perm, :])

    kp = np.arange(128)[:, None]
    qf = np.arange(QCH)[None, :]
    masks = np.stack([(128 * i + kp <= qf) for i in range(4)]).astype(np.float32)

    in_maps = []
    for c in range(N_CORES):
        b, hg = c // 4, c % 4
        csl = slice(hg * 256, (hg + 1) * 256)
        in_maps.append({
            "xT": np.ascontiguousarray(x[b].T),
            "qw": np.ascontiguousarray(qw_p[:, csl]),
            "kw": np.ascontiguousarray(kw_p[:, csl]),
            "vw": np.ascontiguousarray(vw_p[:, csl]),
            "ow": np.ascontiguousarray(ow_p[csl, :]),
            "masks": masks,
        })
    return in_maps


def kernel(x, qw, kw, vw, ow, _trace=False):
    from concourse.bass_utils import run_bass_kernel_spmd

    if _trace:
        _install_ntff_hook()

    nc = _get_nc()
    in_maps = _prep_inputs(x, qw, kw, vw, ow)
    res = run_bass_kernel_spmd(nc, in_maps, core_ids=list(range(N_CORES)),
                               trace=_trace)
    parts = [r["out"] for r in res.results]
    outb = [parts[0] + parts[1] + parts[2] + parts[3],
            parts[4] + parts[5] + parts[6] + parts[7]]
    full = np.stack(outb).astype(np.float32)
    if _trace:
        kernel.last_results = res
        if res.exec_time_ns is not None:
            print(f"HW exec time: {res.exec_time_ns} ns")
        if res.instructions_and_trace:
            print(f"trace: {res.instructions_and_trace[1]}")
    return full


def _install_ntff_hook():
    """The image's antenv lacks axon_hooks; synthesize it so trace=True works."""
    import types
    if 'antenv.axon_hooks' in sys.modules:
        return
    mod = types.ModuleType('antenv.axon_hooks')
    mod._hook = None
    mod.set_axon_ntff_profile_hook = lambda h: setattr(mod, '_hook', h)
    mod.get_axon_ntff_profile_hook = lambda: mod._hook
    sys.modules['antenv.axon_hooks'] = mod
    import antenv
    antenv.axon_hooks = mod
    from trn_agent_boot.trn_boot import _ntff_profile_via_ctypes
    mod.set_axon_ntff_profile_hook(
        _ntff_profile_via_ctypes('/opt/axon/libaxon_pjrt.so'))


# revision 24
# speedup vs baseline: 1.1279x; 1.0259x over previous
"""Multi-head causal attention (B=2, S=2048, D=1024, H=16) on 8 TRN2 cores.

Sharding: core = (batch, group-of-4-heads). Each core computes attention for
its 4 heads of its batch and a rank-256 partial of the output projection;
the host sums the 4 partials per batch. The interleaved head split of the
reference (head h = columns h::16) is undone on the host by permuting the
weight matrices, so on-chip everything is head-contiguous.

On-chip layout (per core, all matmuls in fp32r):
  QT/KT [128, 2048]   head-pair-stacked transposed Q/K (pair p, heads A/B on
                      partitions 0:64 / 64:128)
  S^T   [128, 2, 512] scores for a (k-tile, q-chunk), both heads, one 2-bank
                      PSUM tile; K=64 matmuls row-packed in the PE array
  exp   one ScalarE activation per k-tile over both heads' scores
  PV    oAB[65, 2, 512] += Vaug^T @ P^T; Vaug carries a ones column so row 64
                      accumulates the softmax denominator l
  norm  1/l broadcast via DRAM-roundtrip DMA (stride-0 partition reads are
                      only legal from DRAM), A^T scaled on VectorE
  out   partial = A^T.T @ ow, accumulated over the 2 pairs in PSUM
"""
import sys
sys.path.insert(0, '/opt/trn_rl_repo')

import numpy as np

DIM = 1024
HEADS = 16
S = 2048
B = 2
HD = 64
N_CORES = 8
HPC = 4          # heads per core
PAIRS = 2        # processed as 2 pairs of heads (pair packs the 128-wide PE)
QCH = 512        # q chunk
NKT = S // 128   # k tiles per sequence

_nc_cache = None


def _build(debug=False):
    import concourse.bass as bass
    import concourse.tile as tile
    import concourse.mybir as mybir
    from concourse import bacc
    from concourse.masks import make_identity
    from contextlib import ExitStack

    f32 = mybir.dt.float32
    f32r = mybir.dt.float32r
    Exp = mybir.ActivationFunctionType.Exp

    def bc(ap, n):
        # stride-0 partition broadcast of a [1, ...] DRAM AP to n partitions
        return bass.AP(tensor=ap.tensor, offset=ap.offset,
                       ap=[[0, n]] + [list(d) for d in ap.ap[1:]])

    nc = bacc.Bacc("TRN2", target_bir_lowering=False, debug=False,
                   enable_asserts=False, num_devices=N_CORES)

    xT = nc.dram_tensor("xT", [DIM, S], f32r, kind="ExternalInput").ap()
    qw = nc.dram_tensor("qw", [DIM, 256], f32r, kind="ExternalInput").ap()
    kw = nc.dram_tensor("kw", [DIM, 256], f32r, kind="ExternalInput").ap()
    vw = nc.dram_tensor("vw", [DIM, 256], f32r, kind="ExternalInput").ap()
    ow = nc.dram_tensor("ow", [256, DIM], f32r, kind="ExternalInput").ap()
    masks = nc.dram_tensor("masks", [4, 128, QCH], f32, kind="ExternalInput").ap()
    out = nc.dram_tensor("out", [S, DIM], f32, kind="ExternalOutput").ap()
    if debug:
        dbg = {}
        for name, shape, dt_ in (
                ("dQT0", [128, S], f32r), ("dKT0", [128, S], f32r),
                ("dVT0", [128, S], f32), ("dVaug0", [128, NKT, HD + 1], f32r),
                ("dAT0", [128, S], f32r)):
            dbg[name] = nc.dram_tensor(name, shape, dt_, kind="ExternalOutput").ap()

    with tile.TileContext(nc) as tc, ExitStack() as ctx:
        const_pool = ctx.enter_context(tc.tile_pool(name="const", bufs=1))
        xin_pool = ctx.enter_context(tc.tile_pool(name="xin", bufs=2))
        big_pool = ctx.enter_context(tc.tile_pool(name="big", bufs=1))
        pt_pool = ctx.enter_context(tc.tile_pool(name="pt", bufs=4))
        small_pool = ctx.enter_context(tc.tile_pool(name="small", bufs=4))
        outst_pool = ctx.enter_context(tc.tile_pool(name="outst", bufs=3))
        dram_pool = ctx.enter_context(tc.tile_pool(name="dram", bufs=2, space="DRAM"))
        psum_s = ctx.enter_context(tc.tile_pool(name="psum_s", bufs=2, space="PSUM"))
        psum_o = ctx.enter_context(tc.tile_pool(name="psum_o", bufs=2, space="PSUM"))

        # constants
        qw_sb = const_pool.tile([128, 8, 256], f32r, tag="qw")
        kw_sb = const_pool.tile([128, 8, 256], f32r, tag="kw")
        vw_sb = const_pool.tile([128, 8, 256], f32r, tag="vw")
        nc.sync.dma_start(out=qw_sb, in_=qw.rearrange("(kt p) m -> p kt m", p=128))
        nc.sync.dma_start(out=kw_sb, in_=kw.rearrange("(kt p) m -> p kt m", p=128))
        nc.sync.dma_start(out=vw_sb, in_=vw.rearrange("(kt p) m -> p kt m", p=128))
        ow_sb = const_pool.tile([128, 2, DIM], f32r, tag="ow")
        nc.sync.dma_start(out=ow_sb, in_=ow.rearrange("(t p) n -> p t n", p=128))
        masks_sb = const_pool.tile([128, 4, QCH], f32, tag="masks")
        nc.sync.dma_start(out=masks_sb, in_=masks.rearrange("i p q -> p i q"))
        ident = const_pool.tile([128, 128], f32, tag="ident")
        make_identity(nc, ident)

        QT = [big_pool.tile([128, S], f32r, tag=f"QT{p}", name=f"QT{p}")
              for p in range(PAIRS)]
        KT = [big_pool.tile([128, S], f32r, tag=f"KT{p}", name=f"KT{p}")
              for p in range(PAIRS)]
        VT = [big_pool.tile([128, S], f32, tag=f"VT{p}", name=f"VT{p}")
              for p in range(PAIRS)]
        AT = [big_pool.tile([128, S], f32r, tag=f"AT{p}", name=f"AT{p}")
              for p in range(PAIRS)]
        Vaug = [big_pool.tile([128, NKT, HD + 1], f32r, tag=f"Vaug{h}",
                              name=f"Vaug{h}") for h in range(HPC)]
        ones_c = const_pool.tile([128, NKT, 1], f32, tag="ones")
        nc.vector.memset(ones_c, 1.0)
        for h in range(HPC):
            nc.vector.tensor_copy(out=Vaug[h][:, :, HD:HD + 1], in_=ones_c)

        # ---- Phase 1: QKV projections (QT/KT/VT = W.T @ x.T, head-major) ----
        for j in range(S // QCH):
            qsl = slice(j * QCH, (j + 1) * QCH)
            xt = xin_pool.tile([128, 8, QCH], f32r, tag="xt")
            nc.sync.dma_start(
                out=xt,
                in_=xT.rearrange("(kt p) n -> p kt n", p=128)[:, :, qsl])
            for wsb, dstT in ((qw_sb, QT), (kw_sb, KT), (vw_sb, VT)):
                ps2 = psum_s.tile([128, 2, QCH], f32, tag="s2", name="ps2")
                for ct in range(PAIRS):
                    for kt in range(8):
                        nc.tensor.matmul(
                            ps2[:, ct, :],
                            wsb[:, kt, ct * 128:(ct + 1) * 128],
                            xt[:, kt, :],
                            start=(kt == 0), stop=(kt == 7))
                for ct in range(PAIRS):
                    nc.vector.tensor_copy(out=dstT[ct][:, qsl], in_=ps2[:, ct, :])

        if debug:
            nc.sync.dma_start(out=dbg["dQT0"], in_=QT[0])
            nc.sync.dma_start(out=dbg["dKT0"], in_=KT[0])
            nc.sync.dma_start(out=dbg["dVT0"], in_=VT[0])

        # ---- Phase 1b: V -> Vaug (transpose to [k, hd] + ones column) ----
        for p in range(PAIRS):
            for kt in range(NKT):
                pst = psum_s.tile([128, 128], f32, tag="s2", name="pst")
                nc.tensor.transpose(pst, VT[p][:, kt * 128:(kt + 1) * 128], ident)
                for hh in range(2):
                    nc.vector.tensor_copy(
                        out=Vaug[2 * p + hh][:, kt, 0:HD],
                        in_=pst[:, hh * 64:(hh + 1) * 64])

        if debug:
            nc.sync.dma_start(out=dbg["dVaug0"], in_=Vaug[0])

        # ---- Phase 2: causal attention, pair-packed; then output proj ----
        # Finalize (recip + normalize + out-proj) for chunk i is EMITTED after
        # chunk i+1's first matmuls so the slow DVE reciprocal never
        # head-of-line blocks the PE instruction stream.
        def finalize_norm(p, j, oAB):
            qsl = slice(j * QCH, (j + 1) * QCH)
            rAB = small_pool.tile([33, QCH], f32, tag="r", name="rAB")
            nc.vector.memset(rAB, 0.0)
            with nc.allow_low_precision(reason="recip rows"):
                nc.vector.reciprocal(rAB[0:1, :], oAB[HD:HD + 1, 0, :])
                nc.vector.reciprocal(rAB[32:33, :], oAB[HD:HD + 1, 1, :])
            rd = dram_pool.tile([33, QCH], f32, tag="rd", name="rd")
            nc.sync.dma_start(out=rd, in_=rAB)
            Rsb = small_pool.tile([128, QCH], f32, tag="Rsb", name="Rsb")
            nc.sync.dma_start(out=Rsb[0:64, :], in_=bc(rd[0:1, :], 64))
            nc.sync.dma_start(out=Rsb[64:128, :], in_=bc(rd[32:33, :], 64))
            nc.vector.tensor_copy(out=AT[p][0:64, qsl], in_=oAB[0:HD, 0, :])
            nc.vector.tensor_copy(out=AT[p][64:128, qsl], in_=oAB[0:HD, 1, :])
            nc.vector.tensor_mul(AT[p][:, qsl], AT[p][:, qsl], Rsb)

        def emit_outproj(j):
            # output projection for q rows of chunk j (needs both pairs' AT)
            for rt in range(QCH // 128):
                rsl = slice(j * QCH + rt * 128, j * QCH + (rt + 1) * 128)
                po2 = psum_s.tile([128, 2, 512], f32, tag="s2", name="po2")
                for nch in range(DIM // 512):
                    for pp in range(PAIRS):
                        nc.tensor.matmul(
                            po2[:, nch, :], AT[pp][:, rsl],
                            ow_sb[:, pp, nch * 512:(nch + 1) * 512],
                            start=(pp == 0), stop=(pp == PAIRS - 1))
                ot = outst_pool.tile([128, 2, 512], f32, tag="ot", name="ot")
                nc.vector.tensor_copy(out=ot, in_=po2)
                nc.sync.dma_start(
                    out=out[rsl, :].rearrange("p (a b) -> p a b", a=2),
                    in_=ot)

        pending_norm = None
        pending_proj = None
        for p in range(PAIRS):
            for j in range(S // QCH):
                nkt = 4 * (j + 1)
                qsl = slice(j * QCH, (j + 1) * QCH)
                oAB = psum_o.tile([HD + 1, 2, QCH], f32, tag="o", name="oAB")
                for kt in range(nkt):
                    ksl = slice(kt * 128, (kt + 1) * 128)
                    sAB = psum_s.tile([128, 2, QCH], f32, tag="s2", name="sAB")
                    for hh in range(2):
                        nc.tensor.matmul(sAB[:, hh, :],
                                         KT[p][hh * 64:(hh + 1) * 64, ksl],
                                         QT[p][hh * 64:(hh + 1) * 64, qsl],
                                         start=True, stop=True)
                    pAB = pt_pool.tile([128, 2, QCH], f32r, tag="pt", name="pAB")
                    nc.scalar.activation(out=pAB, in_=sAB, func=Exp)
                    di = kt - 4 * j
                    if di >= 0:  # diagonal tile: apply causal mask
                        for hh in range(2):
                            nc.vector.tensor_mul(pAB[:, hh, :], pAB[:, hh, :],
                                                 masks_sb[:, di, :])
                    for hh in range(2):
                        nc.tensor.matmul(oAB[:, hh, :],
                                         Vaug[2 * p + hh][:, kt, :],
                                         pAB[:, hh, :],
                                         start=(kt == 0), stop=(kt == nkt - 1))
                    if pending_norm is not None and kt == min(3, nkt - 1):
                        finalize_norm(*pending_norm)
                        pending_norm = None
                    if pending_proj is not None and kt == min(7, nkt - 1):
                        emit_outproj(pending_proj)
                        pending_proj = None
                pending_norm = (p, j, oAB)
                if p == PAIRS - 1:
                    pending_proj = j
        finalize_norm(*pending_norm)
        if pending_proj is not None:
            emit_outproj(pending_proj)

        if debug:
            nc.sync.dma_start(out=dbg["dAT0"], in_=AT[0])

    nc.compile()
    return nc


def _get_nc():
    global _nc_cache
    if _nc_cache is None:
        _nc_cache = _build()
    return _nc_cache


def _prep_inputs(x, qw, kw, vw, ow):
    # undo interleaved head split: head h = cols h::16 -> contiguous blocks
    perm = np.concatenate([np.arange(h, DIM, HEADS) for h in range(HEADS)])
    qw_p = (qw[:, perm] / np.float32(np.sqrt(DIM))).astype(np.float32)
    kw_p = np.ascontiguousarray(kw[:, perm])
    vw_p = np.ascontiguousarray(vw[:, perm])
    ow_p = np.ascontiguousarray(ow[perm, :])

    kp = np.arange(128)[:, None]
    qf = np.arange(QCH)[None, :]
    masks = np.stack([(128 * i + kp <= qf) for i in range(4)]).astype(np.float32)

    in_maps = []
    for c in range(N_CORES):
        b, hg = c // 4, c % 4
        csl = slice(hg * 256, (hg + 1) * 256)
        in_maps.append({
            "xT": np.ascontiguousarray(x[b].T),
            "qw": np.ascontiguousarray(qw_p[:, csl]),
            "kw": np.ascontiguousarray(kw_p[:, csl]),
            "vw": np.ascontiguousarray(vw_p[:, csl]),
            "ow": np.ascontiguousarray(ow_p[csl, :]),
            "masks": masks,
        })
    return in_maps


def kernel(x, qw, kw, vw, ow, _trace=False):
    from concourse.bass_utils import run_bass_kernel_spmd

    if _trace:
        _install_ntff_hook()

    nc = _get_nc()
    in_maps = _prep_inputs(x, qw, kw, vw, ow)
    res = run_bass_kernel_spmd(nc, in_maps, core_ids=list(range(N_CORES)),
                               trace=_trace)
    parts = [r["out"] for r in res.results]
    outb = [parts[0] + parts[1] + parts[2] + parts[3],
            parts[4] + parts[5] + parts[6] + parts[7]]
    full = np.stack(outb).astype(np.float32)
    if _trace:
        kernel.last_results = res
        if res.exec_time_ns is not None:
            print(f"HW exec time: {res.exec_time_ns} ns")
        if res.instructions_and_trace:
            print(f"trace: {res.instructions_and_trace[1]}")
    return full


def _install_ntff_hook():
    """The image's antenv lacks axon_hooks; synthesize it so trace=True works."""
    import types
    if 'antenv.axon_hooks' in sys.modules:
        return
    mod = types.ModuleType('antenv.axon_hooks')
    mod._hook = None
    mod.set_axon_ntff_profile_hook = lambda h: setattr(mod, '_hook', h)
    mod.get_axon_ntff_profile_hook = lambda: mod._hook
    sys.modules['antenv.axon_hooks'] = mod
    import antenv
    antenv.axon_hooks = mod
    from trn_agent_boot.trn_boot import _ntff_profile_via_ctypes
    mod.set_axon_ntff_profile_hook(
        _ntff_profile_via_ctypes('/opt/axon/libaxon_pjrt.so'))
